# revision 18
# baseline (speedup 1.0000x reference)
"""Trainium2 Bass kernel for GQA attention (32 q heads / 16 kv heads, head_dim
128, L=2048, D=4608) with RoPE, tanh softcap 50, causal mask, o_proj.

Tensor-parallel over heads across 8 NeuronCores; core c owns q-heads 4c..4c+3
and kv-heads 2c..2c+1; host sums the 8 partial [L, D] outputs.

v5 = v1's projection phase + a restructured attention phase:

Phase 1 (unchanged from v1): one pass over x columns per 512-wide chunk;
512-wide Q/K chains (LDWEIGHTS fully hidden), 256-wide V chains; rope applied
during the PSUM drain.  V stored as plain [k, d] tiles (no ones column).

Phase 2 (new):
  - scores are computed in PAIRS of k-tiles: two 512-wide score MMs land in
    one 2-bank PSUM tile, then a single tanh and a single exp cover
    [128, 1024] - halves the scalar-engine instruction count (its 352-cycle
    per-op overhead was a third of the softcap cost).
  - PV accumulates attnT [d, q] directly (lhsT = V tile, rhs = exp tile,
    512-wide, one PSUM bank per (head, chunk)) - no PE transposes, no
    129-wide matmuls.
  - softmax denominator: gpsimd keeps two running f32 sums of the exp tiles
    (even/odd halves), two ones-vector matmuls reduce them over partitions,
    reciprocal_approx_fast + gpsimd partition-broadcast produce 1/denom,
    folded into the attnT PSUM drain.
  - o_proj groups are interleaved between score pairs with a per-chunk
    budget that defers work toward the later chunks, where the softcap
    chain (tanh+exp grows with chunk index) would otherwise starve the PE.
"""

import numpy as np
import ml_dtypes

import concourse.bass as bass
import concourse.mybir as mybir
import concourse.tile as tile
from concourse import bacc

F32 = mybir.dt.float32
BF16 = mybir.dt.bfloat16
BF16_NP = ml_dtypes.bfloat16
AF = mybir.ActivationFunctionType

N_HEADS = 32
N_KV = 16
HEAD_DIM = 128
ROPE_THETA = 10000.0
SOFTCAP = 50.0
SCALE = 1.0 / 12.0  # 1/sqrt(144)
L = 2048
D = 4608
N_CORES = 8
QH = N_HEADS // N_CORES        # 4 local q heads
KVH = N_KV // N_CORES          # 2 local kv heads
KC = D // 128                  # 36 contraction chunks
NQ = L // 512                  # 4 l-chunks of 512
LT = L // 128                  # 16 l-tiles of 128
DOUT_CHUNKS = D // 512         # 9 o_proj output chunks
PAIR_LAG = 1                   # attnT MM pair trails the exp by this many pairs


def _emit(nc):
    xt_d = nc.dram_tensor("xt", [D, L], BF16, kind="ExternalInput")
    wqt_d = nc.dram_tensor("wqt", [D, QH * 128], BF16, kind="ExternalInput")
    wkt_d = nc.dram_tensor("wkt", [D, KVH * 128], BF16, kind="ExternalInput")
    wvt_d = nc.dram_tensor("wvt", [D, KVH * 128], BF16, kind="ExternalInput")
    wot_d = nc.dram_tensor("wot", [QH * 128, D], BF16, kind="ExternalInput")
    cost_d = nc.dram_tensor("cost", [128, L], BF16, kind="ExternalInput")
    sint_d = nc.dram_tensor("sint", [128, L], BF16, kind="ExternalInput")
    mask_d = nc.dram_tensor("mask", [128, 512], BF16, kind="ExternalInput")
    out_d = nc.dram_tensor("out", [L, D], F32, kind="ExternalOutput")

    with tile.TileContext(nc) as tc:
        with (
            tc.tile_pool(name="const", bufs=1) as const,
            tc.tile_pool(name="persist", bufs=1) as persist,
        ):
            ones = const.tile([128, 1], F32)
            nc.vector.memset(ones[:], 1.0)
            warm = const.tile([128, 2], F32)
            # preload the exp/tanh activation table set during phase 1
            nc.scalar.activation(warm[:, 0:1], ones[:], AF.Tanh)
            nc.scalar.activation(warm[:, 1:2], warm[:, 0:1], AF.Exp)
            cost = const.tile([128, L], BF16)
            sint = const.tile([128, L], BF16)
            mask0 = const.tile([128, 512], BF16)
            nc.sync.dma_start(cost[:], cost_d[:])
            nc.sync.dma_start(sint[:], sint_d[:])
            nc.sync.dma_start(mask0[:], mask_d[:])

            # persistent per-head tensors
            QT = [persist.tile([128, L], BF16, tag=f"qt{h}", name=f"qt{h}") for h in range(QH)]
            KT = [persist.tile([128, L], BF16, tag=f"kt{g}", name=f"kt{g}") for g in range(KVH)]
            V = [persist.tile([128, LT * 128], BF16, tag=f"v{g}", name=f"v{g}") for g in range(KVH)]

            _phase1(nc, tc, cost, sint, QT, KT, V, xt_d, wqt_d, wkt_d, wvt_d)
            _phase2(nc, tc, ones, mask0, QT, KT, V, wot_d, out_d)
    return nc


def _phase1(nc, tc, cost, sint, QT, KT, V, xt_d, wqt_d, wkt_d, wvt_d):
    def drain_rope(rtmp, ps, dst, nq, pj_free):
        """psum [128,512] f32 -> rope -> dst bf16 [128,512] slice."""
        cols = slice(nq * 512, (nq + 1) * 512)
        raw = rtmp.tile([128, 512], F32, tag="raw", name="raw")
        nc.scalar.activation(raw[:], ps[:], AF.Copy)
        swap = rtmp.tile([128, 512], F32, tag="swap", name="swap")
        nc.scalar.activation(swap[0:64, :], ps[64:128, :], AF.Copy)
        nc.scalar.activation(swap[64:128, :], ps[0:64, :], AF.Copy)
        nc.vector.tensor_mul(raw[:], raw[:], cost[:, cols])
        nc.vector.tensor_mul(swap[:], swap[:], sint[:, cols])
        nc.vector.tensor_add(dst[:, cols], raw[:], swap[:])

    with (
        tc.tile_pool(name="xcol", bufs=2) as xcol,
        tc.tile_pool(name="rtmp", bufs=3) as rtmp,
        tc.tile_pool(name="wts", bufs=1) as wts,
        tc.tile_pool(name="pj_psum", bufs=2, space="PSUM") as pj_psum,
    ):
        # single pass over x columns computing Q, K (rope'd, [d, l]) and V
        # (direct [l, d] with xT stationary) per 512-wide chunk.
        wq, wk, wv = [], [], []
        for k in range(KC):
            w = wts.tile([128, QH * 128], BF16, tag=f"q{k}", name=f"wq{k}")
            nc.sync.dma_start(w[:], wqt_d[k * 128:(k + 1) * 128, :])
            wq.append(w)
        xc0 = []
        for k in range(KC):
            t = xcol.tile([128, 512], BF16, tag=f"x{k}", name=f"xc{k}")
            nc.sync.dma_start(t[:], xt_d[k * 128:(k + 1) * 128, 0:512])
            xc0.append(t)
        for k in range(KC):
            w = wts.tile([128, KVH * 128], BF16, tag=f"k{k}", name=f"wk{k}")
            nc.sync.dma_start(w[:], wkt_d[k * 128:(k + 1) * 128, :])
            wk.append(w)
        for k in range(KC):
            w = wts.tile([128, KVH * 128], BF16, tag=f"v{k}", name=f"wv{k}")
            nc.sync.dma_start(w[:], wvt_d[k * 128:(k + 1) * 128, :])
            wv.append(w)

        for nq in range(NQ):
            if nq == 0:
                xc = xc0
            else:
                xc = []
                for k in range(KC):
                    t = xcol.tile([128, 512], BF16, tag=f"x{k}", name=f"xc{k}")
                    nc.sync.dma_start(
                        t[:], xt_d[k * 128:(k + 1) * 128, nq * 512:(nq + 1) * 512])
                    xc.append(t)
            for h in range(QH):
                ps = pj_psum.tile([128, 512], F32, tag="qk", name="ps")
                for k in range(KC):
                    nc.tensor.matmul(
                        ps[:], wq[k][:, h * 128:(h + 1) * 128], xc[k][:],
                        start=(k == 0), stop=(k == KC - 1))
                drain_rope(rtmp, ps, QT[h], nq, pj_psum)
            for g in range(KVH):
                ps = pj_psum.tile([128, 512], F32, tag="qk", name="ps")
                for k in range(KC):
                    nc.tensor.matmul(
                        ps[:], wk[k][:, g * 128:(g + 1) * 128], xc[k][:],
                        start=(k == 0), stop=(k == KC - 1))
                drain_rope(rtmp, ps, KT[g], nq, pj_psum)
            for sub in range(4):
                mk = nq * 4 + sub
                ps = pj_psum.tile([128, KVH * 128], F32, tag="vps", name="ps")
                for k in range(KC):
                    nc.tensor.matmul(
                        ps[:], xc[k][:, sub * 128:(sub + 1) * 128], wv[k][:],
                        start=(k == 0), stop=(k == KC - 1))
                for g in range(KVH):
                    nc.vector.tensor_copy(
                        V[g][:, mk * 128:(mk + 1) * 128],
                        ps[:, g * 128:(g + 1) * 128])


def _phase2(nc, tc, ones, mask0, QT, KT, V, wot_d, out_d):
    with (
        tc.tile_pool(name="wo", bufs=1) as wop,
        tc.tile_pool(name="ttp", bufs=2) as ttp,
        tc.tile_pool(name="ptp", bufs=3) as ptp,
        tc.tile_pool(name="accp", bufs=2) as accp,
        tc.tile_pool(name="atsb", bufs=4) as atsb,
        tc.tile_pool(name="rsb", bufs=2) as rsb,
        tc.tile_pool(name="rbc", bufs=2) as rbc,
        tc.tile_pool(name="ost", bufs=3) as ost,
        tc.tile_pool(name="sc_ps", bufs=2, space="PSUM") as sc_ps,
        tc.tile_pool(name="at_ps", bufs=2, space="PSUM") as at_ps,
        tc.tile_pool(name="op_ps", bufs=2, space="PSUM") as op_ps,
    ):
        WO = []
        for h in range(QH):
            w = wop.tile([128, D], BF16, tag=f"wo{h}")
            nc.sync.dma_start(w[:], wot_d[h * 128:(h + 1) * 128, :])
            WO.append(w)

        at_store = {}

        def build_job(h, r):
            g = h // 2
            nkt = 4 * r + 4
            npair = nkt // 2
            qt = QT[h]
            q0 = r * 512
            state = {"pts": {}, "accA": None, "accB": None, "at": None}

            def att_mm_pair(p):
                pt = state["pts"][p]
                for i in range(2):
                    mk = 2 * p + i
                    nc.tensor.matmul(
                        state["at"][:],
                        V[g][:, mk * 128:(mk + 1) * 128],
                        pt[:, i * 512:(i + 1) * 512],
                        start=(mk == 0), stop=(mk == nkt - 1),
                        skip_group_check=True)

            def pair_unit(p):
                sc = sc_ps.tile([128, 1024], F32, tag="sc", name="sc")
                for i in range(2):
                    mk = 2 * p + i
                    o = mk - 4 * r
                    c0 = max(0, o) * 128
                    # q-aligned within the half so one exp over the whole
                    # pair keeps pt columns aligned with q positions
                    nc.tensor.matmul(sc[:, i * 512 + c0:(i + 1) * 512],
                                     KT[g][:, mk * 128:(mk + 1) * 128],
                                     qt[:, q0 + c0:q0 + 512],
                                     start=True, stop=True)
                # tanh of stale psum in the masked gap is bounded (+-1); its
                # exp is finite and the memset below zeroes it.
                tt = ttp.tile([128, 1024], F32, tag="tt", name="tt")
                nc.scalar.activation(tt[:], sc[:], AF.Tanh, scale=SCALE / SOFTCAP)
                pt = ptp.tile([128, 1024], BF16, tag="pt", name="pt")
                state["pts"][p] = pt
                nc.scalar.activation(pt[:], tt[:], AF.Exp, scale=SOFTCAP)
                for i in range(2):
                    mk = 2 * p + i
                    o = mk - 4 * r
                    c0 = max(0, o) * 128
                    base = i * 512
                    if o >= 0:
                        if c0 > 0:
                            nc.vector.memset(pt[:, base:base + c0], 0.0)
                        nc.vector.tensor_mul(pt[:, base + c0:base + 512],
                                             pt[:, base + c0:base + 512],
                                             mask0[:, 0:512 - c0])
                if p == 0:
                    state["at"] = at_ps.tile([128, 512], F32, tag="at", name="at")
                    state["accA"] = accp.tile([128, 512], F32, tag="accA", name="accA")
                    state["accB"] = accp.tile([128, 512], F32, tag="accB", name="accB")
                    nc.gpsimd.tensor_copy(state["accA"][:], pt[:, 0:512])
                    nc.gpsimd.tensor_copy(state["accB"][:], pt[:, 512:1024])
                else:
                    nc.gpsimd.tensor_add(state["accA"][:], state["accA"][:], pt[:, 0:512])
                    nc.gpsimd.tensor_add(state["accB"][:], state["accB"][:], pt[:, 512:1024])
                if p - PAIR_LAG >= 0:
                    att_mm_pair(p - PAIR_LAG)

            def job_end():
                for p in range(max(0, npair - PAIR_LAG), npair):
                    att_mm_pair(p)
                dn = at_ps.tile([128, 512], F32, tag="at", name="dn")
                nc.tensor.matmul(dn[0:1, 0:512], ones[:], state["accA"][:],
                                 start=True, stop=False)
                nc.tensor.matmul(dn[0:1, 0:512], ones[:], state["accB"][:],
                                 start=False, stop=True)
                rc = rsb.tile([1, 512], F32, tag="rc", name="rc")
                nc.vector.reciprocal_approx_fast(rc[:], dn[0:1, 0:512])
                rb = rbc.tile([128, 512], F32, tag="rb", name="rb")
                nc.gpsimd.partition_broadcast(rb[:], rc[:])
                at_sb = atsb.tile([128, 512], BF16, tag=f"at{h}", name=f"at{h}")
                nc.vector.tensor_mul(at_sb[:], state["at"][:], rb[:])
                at_store[h] = at_sb

            units = [lambda p=p: pair_unit(p) for p in range(npair)]
            units.append(job_end)
            return units

        def oproj_group(r_prev, s, j, at_prev):
            po = op_ps.tile([128, 512], F32, tag="op", name="po")
            for h in range(QH):
                nc.tensor.matmul(
                    po[:], at_prev[h][:, s * 128:(s + 1) * 128],
                    WO[h][:, j * 512:(j + 1) * 512],
                    start=(h == 0), stop=(h == QH - 1))
            ob = ost.tile([128, 512], F32, tag="ob", name="ob")
            nc.vector.tensor_copy(ob[:], po[:])
            row = r_prev * 512 + s * 128
            nc.sync.dma_start(out_d[row:row + 128, j * 512:(j + 1) * 512], ob[:])

        # o_proj work is queued and drawn down with per-chunk budgets that
        # defer it toward later chunks (whose softcap chains are longer).
        c_queue = []
        C_BUDGET = {0: 0, 1: 20, 2: 32, 3: 45}

        for r in range(NQ):
            b_units = []
            for h in range(QH):
                b_units.extend(build_job(h, r))
            if r >= 1:
                at_prev = dict(at_store)
                for s in range(4):
                    for j in range(DOUT_CHUNKS):
                        c_queue.append(
                            lambda r=r, s=s, j=j, ap=at_prev:
                            oproj_group(r - 1, s, j, ap))
            n_c = min(C_BUDGET[r], len(c_queue))
            c_items = c_queue[:n_c]
            del c_queue[:n_c]

            n_slots = max(1, len(b_units) // 2)
            fi = 0
            slot = 0
            for i, u in enumerate(b_units):
                u()
                if i % 2 == 1:
                    slot += 1
                    want = (len(c_items) * slot) // n_slots
                    while fi < want:
                        c_items[fi]()
                        fi += 1
            while fi < len(c_items):
                c_items[fi]()
                fi += 1

        # epilogue: remaining deferred groups + last chunk's o_proj
        for c in c_queue:
            c()
        at_prev = dict(at_store)
        for s in range(4):
            for j in range(DOUT_CHUNKS):
                oproj_group(NQ - 1, s, j, at_prev)


_CACHED_NC = {}


def build(n_iters=1):
    if n_iters not in _CACHED_NC:
        nc = bacc.Bacc("TRN2", target_bir_lowering=False, debug=False)
        _emit(nc)
        nc.compile()
        _CACHED_NC[n_iters] = nc
    return _CACHED_NC[n_iters]


def host_tables():
    inv_freq = 1.0 / (ROPE_THETA ** (np.arange(0, HEAD_DIM, 2, dtype=np.float32) / HEAD_DIM))
    ang = np.arange(L, dtype=np.float32)[:, None] * inv_freq[None, :]  # [L, 64]
    cos, sin = np.cos(ang), np.sin(ang)
    cosT = np.concatenate([cos.T, cos.T], axis=0).astype(BF16_NP)
    sinT = np.concatenate([-sin.T, sin.T], axis=0).astype(BF16_NP)
    return np.ascontiguousarray(cosT), np.ascontiguousarray(sinT)


def host_mask():
    k = np.arange(128)[:, None]
    q = np.arange(512)[None, :]
    return np.ascontiguousarray((q >= k).astype(BF16_NP))


def make_in_maps(x, wq, wk, wv, wo):
    cosT, sinT = host_tables()
    mask = host_mask()
    xt = np.ascontiguousarray(x.reshape(L, D).T).astype(BF16_NP)
    in_maps = []
    for c in range(N_CORES):
        qs = slice(c * QH * 128, (c + 1) * QH * 128)
        kvs = slice(c * KVH * 128, (c + 1) * KVH * 128)
        in_maps.append({
            "xt": xt,
            "wqt": np.ascontiguousarray(wq[qs].T.astype(BF16_NP)),
            "wkt": np.ascontiguousarray(wk[kvs].T.astype(BF16_NP)),
            "wvt": np.ascontiguousarray(wv[kvs].T.astype(BF16_NP)),
            "wot": np.ascontiguousarray(wo[:, qs].T.astype(BF16_NP)),
            "cost": cosT,
            "sint": sinT,
            "mask": mask,
        })
    return in_maps


def run(inputs, trace=False, trace_kwargs=None):
    from concourse.bass_utils import run_bass_kernel_spmd

    nc = build()
    x = np.asarray(inputs["x"], dtype=np.float32)
    in_maps = make_in_maps(
        x,
        np.asarray(inputs["wq"], dtype=np.float32),
        np.asarray(inputs["wk"], dtype=np.float32),
        np.asarray(inputs["wv"], dtype=np.float32),
        np.asarray(inputs["wo"], dtype=np.float32),
    )
    res = run_bass_kernel_spmd(
        nc, in_maps, core_ids=list(range(N_CORES)),
        trace=trace, **(trace_kwargs or {}))
    out = np.zeros((L, D), dtype=np.float32)
    for c in range(N_CORES):
        out += res.results[c]["out"]
    return out.reshape(x.shape), res


def kernel(**inputs) -> np.ndarray:
    out, _ = run(inputs, trace=False)
    return out


# revision 19
# speedup vs baseline: 1.4918x; 1.4918x over previous
"""Trainium2 Bass kernel for GQA attention (32 q heads / 16 kv heads, head_dim
128, L=2048, D=4608) with RoPE, tanh softcap 50, causal mask, o_proj.

Tensor-parallel over heads across 8 NeuronCores; core c owns q-heads 4c..4c+3
and kv-heads 2c..2c+1; host sums the 8 partial [L, D] outputs.

v5 = v1's projection phase + a restructured attention phase:

Phase 1 (unchanged from v1): one pass over x columns per 512-wide chunk;
512-wide Q/K chains (LDWEIGHTS fully hidden), 256-wide V chains; rope applied
during the PSUM drain.  V stored as plain [k, d] tiles (no ones column).

Phase 2 (new):
  - scores are computed in PAIRS of k-tiles: two 512-wide score MMs land in
    one 2-bank PSUM tile, then a single tanh and a single exp cover
    [128, 1024] - halves the scalar-engine instruction count (its 352-cycle
    per-op overhead was a third of the softcap cost).
  - PV accumulates attnT [d, q] directly (lhsT = V tile, rhs = exp tile,
    512-wide, one PSUM bank per (head, chunk)) - no PE transposes, no
    129-wide matmuls.
  - softmax denominator: gpsimd keeps two running f32 sums of the exp tiles
    (even/odd halves), two ones-vector matmuls reduce them over partitions,
    reciprocal_approx_fast + gpsimd partition-broadcast produce 1/denom,
    folded into the attnT PSUM drain.
  - o_proj groups are interleaved between score pairs with a per-chunk
    budget that defers work toward the later chunks, where the softcap
    chain (tanh+exp grows with chunk index) would otherwise starve the PE.
"""

import numpy as np
import ml_dtypes

import concourse.bass as bass
import concourse.mybir as mybir
import concourse.tile as tile
from concourse import bacc

F32 = mybir.dt.float32
BF16 = mybir.dt.bfloat16
BF16_NP = ml_dtypes.bfloat16
AF = mybir.ActivationFunctionType

N_HEADS = 32
N_KV = 16
HEAD_DIM = 128
ROPE_THETA = 10000.0
SOFTCAP = 50.0
SCALE = 1.0 / 12.0  # 1/sqrt(144)
L = 2048
D = 4608
N_CORES = 8
QH = N_HEADS // N_CORES        # 4 local q heads
KVH = N_KV // N_CORES          # 2 local kv heads
KC = D // 128                  # 36 contraction chunks
NQ = L // 512                  # 4 l-chunks of 512
LT = L // 128                  # 16 l-tiles of 128
DOUT_CHUNKS = D // 512         # 9 o_proj output chunks
PAIR_LAG = 1                   # attnT MM pair trails the exp by this many pairs


def _emit(nc):
    xt_d = nc.dram_tensor("xt", [D, L], BF16, kind="ExternalInput")
    wqt_d = nc.dram_tensor("wqt", [D, QH * 128], BF16, kind="ExternalInput")
    wkt_d = nc.dram_tensor("wkt", [D, KVH * 128], BF16, kind="ExternalInput")
    wvt_d = nc.dram_tensor("wvt", [D, KVH * 128], BF16, kind="ExternalInput")
    wot_d = nc.dram_tensor("wot", [QH * 128, D], BF16, kind="ExternalInput")
    cost_d = nc.dram_tensor("cost", [128, L], BF16, kind="ExternalInput")
    sint_d = nc.dram_tensor("sint", [128, L], BF16, kind="ExternalInput")
    mask_d = nc.dram_tensor("mask", [128, 512], BF16, kind="ExternalInput")
    out_d = nc.dram_tensor("out", [L, D], F32, kind="ExternalOutput")

    with tile.TileContext(nc) as tc:
        with (
            tc.tile_pool(name="const", bufs=1) as const,
            tc.tile_pool(name="persist", bufs=1) as persist,
        ):
            ones = const.tile([128, 1], F32)
            nc.vector.memset(ones[:], 1.0)
            warm = const.tile([128, 2], F32)
            # preload the exp/tanh activation table set during phase 1
            nc.scalar.activation(warm[:, 0:1], ones[:], AF.Tanh)
            nc.scalar.activation(warm[:, 1:2], warm[:, 0:1], AF.Exp)
            cost = const.tile([128, L], BF16)
            sint = const.tile([128, L], BF16)
            mask0 = const.tile([128, 512], BF16)
            nc.sync.dma_start(cost[:], cost_d[:])
            nc.sync.dma_start(sint[:], sint_d[:])
            nc.sync.dma_start(mask0[:], mask_d[:])

            # persistent per-head tensors
            QT = [persist.tile([128, L], BF16, tag=f"qt{h}", name=f"qt{h}") for h in range(QH)]
            KT = [persist.tile([128, L], BF16, tag=f"kt{g}", name=f"kt{g}") for g in range(KVH)]
            # V extended with a ones column per k-tile: [128, 16*129]
            V = [persist.tile([128, LT * 129], BF16, tag=f"v{g}", name=f"v{g}") for g in range(KVH)]

            _phase1(nc, tc, cost, sint, QT, KT, V, xt_d, wqt_d, wkt_d, wvt_d)
            _phase2(nc, tc, ones, mask0, QT, KT, V, wot_d, out_d)
    return nc


def _phase1(nc, tc, cost, sint, QT, KT, V, xt_d, wqt_d, wkt_d, wvt_d):
    def drain_rope(rtmp, ps, dst, nq, pj_free):
        """psum [128,512] f32 -> rope -> dst bf16 [128,512] slice."""
        cols = slice(nq * 512, (nq + 1) * 512)
        raw = rtmp.tile([128, 512], F32, tag="raw", name="raw")
        nc.scalar.activation(raw[:], ps[:], AF.Copy)
        swap = rtmp.tile([128, 512], F32, tag="swap", name="swap")
        nc.scalar.activation(swap[0:64, :], ps[64:128, :], AF.Copy)
        nc.scalar.activation(swap[64:128, :], ps[0:64, :], AF.Copy)
        nc.vector.tensor_mul(raw[:], raw[:], cost[:, cols])
        nc.vector.tensor_mul(swap[:], swap[:], sint[:, cols])
        nc.vector.tensor_add(dst[:, cols], raw[:], swap[:])

    with (
        tc.tile_pool(name="xcol", bufs=2) as xcol,
        tc.tile_pool(name="rtmp", bufs=3) as rtmp,
        tc.tile_pool(name="wts", bufs=1) as wts,
        tc.tile_pool(name="pj_psum", bufs=2, space="PSUM") as pj_psum,
    ):
        # single pass over x columns computing Q, K (rope'd, [d, l]) and V
        # (direct [l, d] with xT stationary) per 512-wide chunk.
        wq, wk, wv = [], [], []
        for k in range(KC):
            w = wts.tile([128, QH * 128], BF16, tag=f"q{k}", name=f"wq{k}")
            nc.sync.dma_start(w[:], wqt_d[k * 128:(k + 1) * 128, :])
            wq.append(w)
        xc0 = []
        for k in range(KC):
            t = xcol.tile([128, 512], BF16, tag=f"x{k}", name=f"xc{k}")
            nc.sync.dma_start(t[:], xt_d[k * 128:(k + 1) * 128, 0:512])
            xc0.append(t)
        for k in range(KC):
            w = wts.tile([128, KVH * 128], BF16, tag=f"k{k}", name=f"wk{k}")
            nc.sync.dma_start(w[:], wkt_d[k * 128:(k + 1) * 128, :])
            wk.append(w)
        for k in range(KC):
            w = wts.tile([128, KVH * 128], BF16, tag=f"v{k}", name=f"wv{k}")
            nc.sync.dma_start(w[:], wvt_d[k * 128:(k + 1) * 128, :])
            wv.append(w)

        for nq in range(NQ):
            if nq == 0:
                xc = xc0
            else:
                xc = []
                for k in range(KC):
                    t = xcol.tile([128, 512], BF16, tag=f"x{k}", name=f"xc{k}")
                    nc.sync.dma_start(
                        t[:], xt_d[k * 128:(k + 1) * 128, nq * 512:(nq + 1) * 512])
                    xc.append(t)
            for h in range(QH):
                ps = pj_psum.tile([128, 512], F32, tag="qk", name="ps")
                for k in range(KC):
                    nc.tensor.matmul(
                        ps[:], wq[k][:, h * 128:(h + 1) * 128], xc[k][:],
                        start=(k == 0), stop=(k == KC - 1))
                drain_rope(rtmp, ps, QT[h], nq, pj_psum)
            for g in range(KVH):
                ps = pj_psum.tile([128, 512], F32, tag="qk", name="ps")
                for k in range(KC):
                    nc.tensor.matmul(
                        ps[:], wk[k][:, g * 128:(g + 1) * 128], xc[k][:],
                        start=(k == 0), stop=(k == KC - 1))
                drain_rope(rtmp, ps, KT[g], nq, pj_psum)
            for sub in range(4):
                mk = nq * 4 + sub
                ps = pj_psum.tile([128, KVH * 128], F32, tag="vps", name="ps")
                for k in range(KC):
                    nc.tensor.matmul(
                        ps[:], xc[k][:, sub * 128:(sub + 1) * 128], wv[k][:],
                        start=(k == 0), stop=(k == KC - 1))
                for g in range(KVH):
                    nc.vector.tensor_copy(
                        V[g][:, mk * 129:mk * 129 + 128],
                        ps[:, g * 128:(g + 1) * 128])
                    nc.vector.memset(
                        V[g][:, mk * 129 + 128:mk * 129 + 129], 1.0)


def _phase2(nc, tc, ones, mask0, QT, KT, VE, wot_d, out_d):
    from concourse.masks import make_identity

    with (
        tc.tile_pool(name="wo", bufs=1) as wop,
        tc.tile_pool(name="ident", bufs=1) as idp,
        tc.tile_pool(name="ttp", bufs=2) as ttp,
        tc.tile_pool(name="ptp", bufs=16) as ptp,
        tc.tile_pool(name="attnt", bufs=3) as attp,
        tc.tile_pool(name="small", bufs=4) as small,
        tc.tile_pool(name="ost", bufs=3) as ost,
        tc.tile_pool(name="sc_ps", bufs=1, space="PSUM") as sc_ps,
        tc.tile_pool(name="pv_ps", bufs=2, space="PSUM") as pv_ps,
        tc.tile_pool(name="atr_ps", bufs=2, space="PSUM") as atr_ps,
        tc.tile_pool(name="op_ps", bufs=2, space="PSUM") as op_ps,
    ):
        ident = idp.tile([128, 128], BF16)
        make_identity(nc, ident[:])
        WO = []
        for h in range(QH):
            w = wop.tile([128, D], BF16, tag=f"wo{h}")
            nc.sync.dma_start(w[:], wot_d[h * 128:(h + 1) * 128, :])
            WO.append(w)

        at_store = {}

        def make_job(h, r):
            """Closures for one (head, chunk): score pairs then 4 PV units."""
            g = h // 2
            nkt = 4 * r + 4
            npair = nkt // 2
            qt = QT[h]
            q0 = r * 512
            pts = {}

            def pair_unit(p):
                sc = sc_ps.tile([128, 1024], F32, tag="sc", name="sc")
                for i in range(2):
                    mk = 2 * p + i
                    o = mk - 4 * r
                    c0 = max(0, o) * 128
                    nc.tensor.matmul(sc[:, i * 512 + c0:(i + 1) * 512],
                                     KT[g][:, mk * 128:(mk + 1) * 128],
                                     qt[:, q0 + c0:q0 + 512],
                                     start=True, stop=True)
                # tanh of stale psum in masked gaps is bounded; exp of it is
                # finite and the memset below zeroes it.
                tt = ttp.tile([128, 1024], F32, tag="tt", name="tt")
                nc.scalar.activation(tt[:], sc[:], AF.Tanh, scale=SCALE / SOFTCAP)
                pt = ptp.tile([128, 1024], BF16, tag="pt", name="pt")
                pts[p] = pt
                nc.scalar.activation(pt[:], tt[:], AF.Exp, scale=SOFTCAP)
                for i in range(2):
                    mk = 2 * p + i
                    o = mk - 4 * r
                    c0 = max(0, o) * 128
                    base = i * 512
                    if o >= 0:
                        if c0 > 0:
                            nc.vector.memset(pt[:, base:base + c0], 0.0)
                        nc.vector.tensor_mul(pt[:, base + c0:base + 512],
                                             pt[:, base + c0:base + 512],
                                             mask0[:, 0:512 - c0])

            def pv_unit(s):
                # attn for q-rows [s*128, (s+1)*128): 129-wide PV accumulation
                # (col 128 of VE is ones -> softmax denominator for free)
                nks = 4 * r + s + 1
                pv = pv_ps.tile([128, 129], F32, tag="pv", name="pv")
                for mk in range(nks):
                    nc.tensor.matmul(
                        pv[:], pts[mk // 2][:, (mk % 2) * 512 + s * 128:
                                            (mk % 2) * 512 + (s + 1) * 128],
                        VE[g][:, mk * 129:(mk + 1) * 129],
                        start=(mk == 0), stop=(mk == nks - 1))
                recip = small.tile([128, 1], F32, tag="recip", name="recip")
                nc.vector.reciprocal(recip[:], pv[:, 128:129])
                attn_q = small.tile([128, 128], BF16, tag="attnq", name="attnq")
                nc.vector.tensor_scalar_mul(attn_q[:], pv[:, 0:128], recip[:])
                tp = atr_ps.tile([128, 128], BF16, tag="atr", name="tp")
                nc.tensor.transpose(tp[:], attn_q[:], ident[:])
                nc.vector.tensor_copy(at_store[h][:, s * 128:(s + 1) * 128], tp[:])

            def start_pv():
                at_store[h] = attp.tile([128, 512], BF16, tag=f"at{h}", name=f"at{h}")

            units = [lambda p=p: pair_unit(p) for p in range(npair)]
            pv_units = [start_pv] + [lambda s=s: pv_unit(s) for s in range(4)]
            return units, pv_units

        def oproj_group(r_prev, s, j, at_prev):
            po = op_ps.tile([128, 512], F32, tag="op", name="po")
            for h in range(QH):
                nc.tensor.matmul(
                    po[:], at_prev[h][:, s * 128:(s + 1) * 128],
                    WO[h][:, j * 512:(j + 1) * 512],
                    start=(h == 0), stop=(h == QH - 1))
            ob = ost.tile([128, 512], F32, tag="ob", name="ob")
            nc.vector.tensor_copy(ob[:], po[:])
            row = r_prev * 512 + s * 128
            nc.sync.dma_start(out_d[row:row + 128, j * 512:(j + 1) * 512], ob[:])

        # o_proj deferred toward later chunks (longer softcap chains there)
        c_queue = []
        C_BUDGET = {0: 0, 1: 20, 2: 32, 3: 45}

        for r in range(NQ):
            # B stream: score pairs of head h interleaved with PV of head h-1
            b_units = []
            prev_pv = []
            for h in range(QH):
                units, pv_units = make_job(h, r)
                merged = []
                n = max(len(units), len(prev_pv))
                for i in range(n):
                    if i < len(units):
                        merged.append(units[i])
                    if i < len(prev_pv):
                        merged.append(prev_pv[i])
                b_units.extend(merged)
                prev_pv = pv_units
            b_units.extend(prev_pv)  # PV of the last head

            if r >= 1:
                at_prev = dict(at_store)
                for s in range(4):
                    for j in range(DOUT_CHUNKS):
                        c_queue.append(
                            lambda r=r, s=s, j=j, ap=at_prev:
                            oproj_group(r - 1, s, j, ap))
            n_c = min(C_BUDGET[r], len(c_queue))
            c_items = c_queue[:n_c]
            del c_queue[:n_c]

            n_slots = max(1, len(b_units) // 2)
            fi = 0
            slot = 0
            for i, u in enumerate(b_units):
                u()
                if i % 2 == 1:
                    slot += 1
                    want = (len(c_items) * slot) // n_slots
                    while fi < want:
                        c_items[fi]()
                        fi += 1
            while fi < len(c_items):
                c_items[fi]()
                fi += 1

        for c in c_queue:
            c()
        at_prev = dict(at_store)
        for s in range(4):
            for j in range(DOUT_CHUNKS):
                oproj_group(NQ - 1, s, j, at_prev)


_CACHED_NC = {}


def build(n_iters=1):
    if n_iters not in _CACHED_NC:
        nc = bacc.Bacc("TRN2", target_bir_lowering=False, debug=False)
        _emit(nc)
        nc.compile()
        _CACHED_NC[n_iters] = nc
    return _CACHED_NC[n_iters]


def host_tables():
    inv_freq = 1.0 / (ROPE_THETA ** (np.arange(0, HEAD_DIM, 2, dtype=np.float32) / HEAD_DIM))
    ang = np.arange(L, dtype=np.float32)[:, None] * inv_freq[None, :]  # [L, 64]
    cos, sin = np.cos(ang), np.sin(ang)
    cosT = np.concatenate([cos.T, cos.T], axis=0).astype(BF16_NP)
    sinT = np.concatenate([-sin.T, sin.T], axis=0).astype(BF16_NP)
    return np.ascontiguousarray(cosT), np.ascontiguousarray(sinT)


def host_mask():
    k = np.arange(128)[:, None]
    q = np.arange(512)[None, :]
    return np.ascontiguousarray((q >= k).astype(BF16_NP))


def make_in_maps(x, wq, wk, wv, wo):
    cosT, sinT = host_tables()
    mask = host_mask()
    xt = np.ascontiguousarray(x.reshape(L, D).T).astype(BF16_NP)
    in_maps = []
    for c in range(N_CORES):
        qs = slice(c * QH * 128, (c + 1) * QH * 128)
        kvs = slice(c * KVH * 128, (c + 1) * KVH * 128)
        in_maps.append({
            "xt": xt,
            "wqt": np.ascontiguousarray(wq[qs].T.astype(BF16_NP)),
            "wkt": np.ascontiguousarray(wk[kvs].T.astype(BF16_NP)),
            "wvt": np.ascontiguousarray(wv[kvs].T.astype(BF16_NP)),
            "wot": np.ascontiguousarray(wo[:, qs].T.astype(BF16_NP)),
            "cost": cosT,
            "sint": sinT,
            "mask": mask,
        })
    return in_maps


def run(inputs, trace=False, trace_kwargs=None):
    from concourse.bass_utils import run_bass_kernel_spmd

    nc = build()
    x = np.asarray(inputs["x"], dtype=np.float32)
    in_maps = make_in_maps(
        x,
        np.asarray(inputs["wq"], dtype=np.float32),
        np.asarray(inputs["wk"], dtype=np.float32),
        np.asarray(inputs["wv"], dtype=np.float32),
        np.asarray(inputs["wo"], dtype=np.float32),
    )
    res = run_bass_kernel_spmd(
        nc, in_maps, core_ids=list(range(N_CORES)),
        trace=trace, **(trace_kwargs or {}))
    out = np.zeros((L, D), dtype=np.float32)
    for c in range(N_CORES):
        out += res.results[c]["out"]
    return out.reshape(x.shape), res


def kernel(**inputs) -> np.ndarray:
    out, _ = run(inputs, trace=False)
    return out


# revision 21
# speedup vs baseline: 1.4923x; 1.0003x over previous
"""Trainium2 Bass kernel for GQA attention (32 q heads / 16 kv heads, head_dim
128, L=2048, D=4608) with RoPE, tanh softcap 50, causal mask, o_proj.

Tensor-parallel over heads across 8 NeuronCores; core c owns q-heads 4c..4c+3
and kv-heads 2c..2c+1; host sums the 8 partial [L, D] outputs.

v5 = v1's projection phase + a restructured attention phase:

Phase 1 (unchanged from v1): one pass over x columns per 512-wide chunk;
512-wide Q/K chains (LDWEIGHTS fully hidden), 256-wide V chains; rope applied
during the PSUM drain.  V stored as plain [k, d] tiles (no ones column).

Phase 2 (new):
  - scores are computed in PAIRS of k-tiles: two 512-wide score MMs land in
    one 2-bank PSUM tile, then a single tanh and a single exp cover
    [128, 1024] - halves the scalar-engine instruction count (its 352-cycle
    per-op overhead was a third of the softcap cost).
  - PV accumulates attnT [d, q] directly (lhsT = V tile, rhs = exp tile,
    512-wide, one PSUM bank per (head, chunk)) - no PE transposes, no
    129-wide matmuls.
  - softmax denominator: gpsimd keeps two running f32 sums of the exp tiles
    (even/odd halves), two ones-vector matmuls reduce them over partitions,
    reciprocal_approx_fast + gpsimd partition-broadcast produce 1/denom,
    folded into the attnT PSUM drain.
  - o_proj groups are interleaved between score pairs with a per-chunk
    budget that defers work toward the later chunks, where the softcap
    chain (tanh+exp grows with chunk index) would otherwise starve the PE.
"""

import numpy as np
import ml_dtypes

import concourse.bass as bass
import concourse.mybir as mybir
import concourse.tile as tile
from concourse import bacc

F32 = mybir.dt.float32
BF16 = mybir.dt.bfloat16
BF16_NP = ml_dtypes.bfloat16
AF = mybir.ActivationFunctionType

N_HEADS = 32
N_KV = 16
HEAD_DIM = 128
ROPE_THETA = 10000.0
SOFTCAP = 50.0
SCALE = 1.0 / 12.0  # 1/sqrt(144)
L = 2048
D = 4608
N_CORES = 8
QH = N_HEADS // N_CORES        # 4 local q heads
KVH = N_KV // N_CORES          # 2 local kv heads
KC = D // 128                  # 36 contraction chunks
NQ = L // 512                  # 4 l-chunks of 512
LT = L // 128                  # 16 l-tiles of 128
DOUT_CHUNKS = D // 512         # 9 o_proj output chunks
PAIR_LAG = 1                   # attnT MM pair trails the exp by this many pairs


def _emit(nc):
    xt_d = nc.dram_tensor("xt", [D, L], BF16, kind="ExternalInput")
    wqt_d = nc.dram_tensor("wqt", [D, QH * 128], BF16, kind="ExternalInput")
    wkt_d = nc.dram_tensor("wkt", [D, KVH * 128], BF16, kind="ExternalInput")
    wvt_d = nc.dram_tensor("wvt", [D, KVH * 128], BF16, kind="ExternalInput")
    wot_d = nc.dram_tensor("wot", [QH * 128, D], BF16, kind="ExternalInput")
    cost_d = nc.dram_tensor("cost", [128, L], BF16, kind="ExternalInput")
    sint_d = nc.dram_tensor("sint", [128, L], BF16, kind="ExternalInput")
    mask_d = nc.dram_tensor("mask", [128, 512], BF16, kind="ExternalInput")
    out_d = nc.dram_tensor("out", [L, D], F32, kind="ExternalOutput")

    with tile.TileContext(nc) as tc:
        with (
            tc.tile_pool(name="const", bufs=1) as const,
            tc.tile_pool(name="persist", bufs=1) as persist,
        ):
            ones = const.tile([128, 1], F32)
            nc.vector.memset(ones[:], 1.0)
            warm = const.tile([128, 2], F32)
            # preload the exp/tanh activation table set during phase 1
            nc.scalar.activation(warm[:, 0:1], ones[:], AF.Tanh)
            nc.scalar.activation(warm[:, 1:2], warm[:, 0:1], AF.Exp)
            cost = const.tile([128, L], BF16)
            sint = const.tile([128, L], BF16)
            mask0 = const.tile([128, 512], BF16)

            # persistent per-head tensors
            QT = [persist.tile([128, L], BF16, tag=f"qt{h}", name=f"qt{h}") for h in range(QH)]
            KT = [persist.tile([128, L], BF16, tag=f"kt{g}", name=f"kt{g}") for g in range(KVH)]
            # V extended with a ones column per k-tile: [128, 16*129]
            V = [persist.tile([128, LT * 129], BF16, tag=f"v{g}", name=f"v{g}") for g in range(KVH)]

            _phase1(nc, tc, cost, sint, mask0,
                    dict(xt=xt_d, wqt=wqt_d, wkt=wkt_d, wvt=wvt_d,
                         cost=cost_d, sint=sint_d, mask=mask_d), QT, KT, V)
            _phase2(nc, tc, ones, mask0, QT, KT, V, wot_d, out_d)
    return nc


def _phase1(nc, tc, cost, sint, mask0, dram, QT, KT, V):
    def drain_rope(rtmp, ps, dst, nq, pj_free):
        """psum [128,512] f32 -> rope -> dst bf16 [128,512] slice."""
        cols = slice(nq * 512, (nq + 1) * 512)
        raw = rtmp.tile([128, 512], F32, tag="raw", name="raw")
        nc.scalar.activation(raw[:], ps[:], AF.Copy)
        swap = rtmp.tile([128, 512], F32, tag="swap", name="swap")
        nc.scalar.activation(swap[0:64, :], ps[64:128, :], AF.Copy)
        nc.scalar.activation(swap[64:128, :], ps[0:64, :], AF.Copy)
        nc.vector.tensor_mul(raw[:], raw[:], cost[:, cols])
        nc.vector.tensor_mul(swap[:], swap[:], sint[:, cols])
        nc.vector.tensor_add(dst[:, cols], raw[:], swap[:])

    with (
        tc.tile_pool(name="xcol", bufs=2) as xcol,
        tc.tile_pool(name="rtmp", bufs=3) as rtmp,
        tc.tile_pool(name="wts", bufs=1) as wts,
        tc.tile_pool(name="pj_psum", bufs=2, space="PSUM") as pj_psum,
    ):
        # single pass over x columns computing Q, K (rope'd, [d, l]) and V
        # (direct [l, d] with xT stationary) per 512-wide chunk.
        wq, wk, wv, xc0 = [], [], [], []
        for k in range(KC):
            w = wts.tile([128, QH * 128], BF16, tag=f"q{k}", name=f"wq{k}")
            nc.sync.dma_start(w[:], dram["wqt"][k * 128:(k + 1) * 128, :])
            wq.append(w)
            t = xcol.tile([128, 512], BF16, tag=f"x{k}", name=f"xc{k}")
            nc.sync.dma_start(t[:], dram["xt"][k * 128:(k + 1) * 128, 0:512])
            xc0.append(t)
        nc.sync.dma_start(cost[:], dram["cost"][:])
        nc.sync.dma_start(sint[:], dram["sint"][:])
        for k in range(KC):
            w = wts.tile([128, KVH * 128], BF16, tag=f"k{k}", name=f"wk{k}")
            nc.sync.dma_start(w[:], dram["wkt"][k * 128:(k + 1) * 128, :])
            wk.append(w)
        for k in range(KC):
            w = wts.tile([128, KVH * 128], BF16, tag=f"v{k}", name=f"wv{k}")
            nc.sync.dma_start(w[:], dram["wvt"][k * 128:(k + 1) * 128, :])
            wv.append(w)
        nc.sync.dma_start(mask0[:], dram["mask"][:])

        for nq in range(NQ):
            if nq == 0:
                xc = xc0
            else:
                xc = []
                for k in range(KC):
                    t = xcol.tile([128, 512], BF16, tag=f"x{k}", name=f"xc{k}")
                    nc.sync.dma_start(
                        t[:], dram["xt"][k * 128:(k + 1) * 128, nq * 512:(nq + 1) * 512])
                    xc.append(t)
            for h in range(QH):
                ps = pj_psum.tile([128, 512], F32, tag="qk", name="ps")
                for k in range(KC):
                    nc.tensor.matmul(
                        ps[:], wq[k][:, h * 128:(h + 1) * 128], xc[k][:],
                        start=(k == 0), stop=(k == KC - 1))
                drain_rope(rtmp, ps, QT[h], nq, pj_psum)
            for g in range(KVH):
                ps = pj_psum.tile([128, 512], F32, tag="qk", name="ps")
                for k in range(KC):
                    nc.tensor.matmul(
                        ps[:], wk[k][:, g * 128:(g + 1) * 128], xc[k][:],
                        start=(k == 0), stop=(k == KC - 1))
                drain_rope(rtmp, ps, KT[g], nq, pj_psum)
            for sub in range(4):
                mk = nq * 4 + sub
                ps = pj_psum.tile([128, KVH * 128], F32, tag="vps", name="ps")
                for k in range(KC):
                    nc.tensor.matmul(
                        ps[:], xc[k][:, sub * 128:(sub + 1) * 128], wv[k][:],
                        start=(k == 0), stop=(k == KC - 1))
                for g in range(KVH):
                    nc.vector.tensor_copy(
                        V[g][:, mk * 129:mk * 129 + 128],
                        ps[:, g * 128:(g + 1) * 128])
                    nc.vector.memset(
                        V[g][:, mk * 129 + 128:mk * 129 + 129], 1.0)


def _phase2(nc, tc, ones, mask0, QT, KT, VE, wot_d, out_d):
    from concourse.masks import make_identity

    with (
        tc.tile_pool(name="wo", bufs=1) as wop,
        tc.tile_pool(name="ident", bufs=1) as idp,
        tc.tile_pool(name="ttp", bufs=2) as ttp,
        tc.tile_pool(name="ptp", bufs=16) as ptp,
        tc.tile_pool(name="attnt", bufs=3) as attp,
        tc.tile_pool(name="small", bufs=4) as small,
        tc.tile_pool(name="ost", bufs=3) as ost,
        tc.tile_pool(name="sc_ps", bufs=1, space="PSUM") as sc_ps,
        tc.tile_pool(name="pv_ps", bufs=2, space="PSUM") as pv_ps,
        tc.tile_pool(name="atr_ps", bufs=2, space="PSUM") as atr_ps,
        tc.tile_pool(name="op_ps", bufs=2, space="PSUM") as op_ps,
    ):
        ident = idp.tile([128, 128], BF16)
        make_identity(nc, ident[:])
        WO = []
        for h in range(QH):
            w = wop.tile([128, D], BF16, tag=f"wo{h}")
            nc.sync.dma_start(w[:], wot_d[h * 128:(h + 1) * 128, :])
            WO.append(w)

        at_store = {}

        def make_job(h, r):
            """Closures for one (head, chunk): score pairs then 4 PV units."""
            g = h // 2
            nkt = 4 * r + 4
            npair = nkt // 2
            qt = QT[h]
            q0 = r * 512
            pts = {}

            def pair_unit(p):
                sc = sc_ps.tile([128, 1024], F32, tag="sc", name="sc")
                for i in range(2):
                    mk = 2 * p + i
                    o = mk - 4 * r
                    c0 = max(0, o) * 128
                    nc.tensor.matmul(sc[:, i * 512 + c0:(i + 1) * 512],
                                     KT[g][:, mk * 128:(mk + 1) * 128],
                                     qt[:, q0 + c0:q0 + 512],
                                     start=True, stop=True)
                # tanh of stale psum in masked gaps is bounded; exp of it is
                # finite and the memset below zeroes it.
                tt = ttp.tile([128, 1024], F32, tag="tt", name="tt")
                nc.scalar.activation(tt[:], sc[:], AF.Tanh, scale=SCALE / SOFTCAP)
                pt = ptp.tile([128, 1024], BF16, tag="pt", name="pt")
                pts[p] = pt
                nc.scalar.activation(pt[:], tt[:], AF.Exp, scale=SOFTCAP)
                for i in range(2):
                    mk = 2 * p + i
                    o = mk - 4 * r
                    c0 = max(0, o) * 128
                    base = i * 512
                    if o >= 0:
                        if c0 > 0:
                            nc.vector.memset(pt[:, base:base + c0], 0.0)
                        nc.vector.tensor_mul(pt[:, base + c0:base + 512],
                                             pt[:, base + c0:base + 512],
                                             mask0[:, 0:512 - c0])

            def pv_unit(s):
                # attn for q-rows [s*128, (s+1)*128): 129-wide PV accumulation
                # (col 128 of VE is ones -> softmax denominator for free)
                nks = 4 * r + s + 1
                pv = pv_ps.tile([128, 129], F32, tag="pv", name="pv")
                for mk in range(nks):
                    nc.tensor.matmul(
                        pv[:], pts[mk // 2][:, (mk % 2) * 512 + s * 128:
                                            (mk % 2) * 512 + (s + 1) * 128],
                        VE[g][:, mk * 129:(mk + 1) * 129],
                        start=(mk == 0), stop=(mk == nks - 1))
                recip = small.tile([128, 1], F32, tag="recip", name="recip")
                nc.vector.reciprocal(recip[:], pv[:, 128:129])
                attn_q = small.tile([128, 128], BF16, tag="attnq", name="attnq")
                nc.vector.tensor_scalar_mul(attn_q[:], pv[:, 0:128], recip[:])
                tp = atr_ps.tile([128, 128], BF16, tag="atr", name="tp")
                nc.tensor.transpose(tp[:], attn_q[:], ident[:])
                nc.vector.tensor_copy(at_store[h][:, s * 128:(s + 1) * 128], tp[:])

            def start_pv():
                at_store[h] = attp.tile([128, 512], BF16, tag=f"at{h}", name=f"at{h}")

            units = [lambda p=p: pair_unit(p) for p in range(npair)]
            pv_units = [start_pv] + [lambda s=s: pv_unit(s) for s in range(4)]
            return units, pv_units

        def oproj_group(r_prev, s, j, at_prev, drain="v"):
            po = op_ps.tile([128, 512], F32, tag="op", name="po")
            for h in range(QH):
                nc.tensor.matmul(
                    po[:], at_prev[h][:, s * 128:(s + 1) * 128],
                    WO[h][:, j * 512:(j + 1) * 512],
                    start=(h == 0), stop=(h == QH - 1))
            ob = ost.tile([128, 512], F32, tag="ob", name="ob")
            if drain == "s":
                nc.scalar.copy(ob[:], po[:])
            else:
                nc.vector.tensor_copy(ob[:], po[:])
            row = r_prev * 512 + s * 128
            nc.sync.dma_start(out_d[row:row + 128, j * 512:(j + 1) * 512], ob[:])

        # o_proj deferred toward later chunks (longer softcap chains there)
        c_queue = []
        C_BUDGET = {0: 0, 1: 20, 2: 32, 3: 10 ** 6}

        for r in range(NQ):
            # B stream: score pairs of head h interleaved with PV of head h-1
            b_units = []
            prev_pv = []
            for h in range(QH):
                units, pv_units = make_job(h, r)
                merged = []
                n = max(len(units), len(prev_pv))
                for i in range(n):
                    if i < len(units):
                        merged.append(units[i])
                    if i < len(prev_pv):
                        merged.append(prev_pv[i])
                b_units.extend(merged)
                prev_pv = pv_units
            b_units.extend(prev_pv)  # PV of the last head

            if r >= 1:
                at_prev = dict(at_store)
                for s in range(4):
                    for j in range(DOUT_CHUNKS):
                        c_queue.append(
                            lambda r=r, s=s, j=j, ap=at_prev:
                            oproj_group(r - 1, s, j, ap))
            n_c = min(C_BUDGET[r], len(c_queue))
            c_items = c_queue[:n_c]
            del c_queue[:n_c]

            n_slots = max(1, len(b_units) // 2)
            fi = 0
            slot = 0
            for i, u in enumerate(b_units):
                u()
                if i % 2 == 1:
                    slot += 1
                    want = (len(c_items) * slot) // n_slots
                    while fi < want:
                        c_items[fi]()
                        fi += 1
            while fi < len(c_items):
                c_items[fi]()
                fi += 1

        for c in c_queue:
            c()
        at_prev = dict(at_store)
        for s in range(4):
            for j in range(DOUT_CHUNKS):
                oproj_group(NQ - 1, s, j, at_prev, drain="s" if j % 2 else "v")


_CACHED_NC = {}


def build(n_iters=1):
    if n_iters not in _CACHED_NC:
        nc = bacc.Bacc("TRN2", target_bir_lowering=False, debug=False)
        _emit(nc)
        nc.compile()
        _CACHED_NC[n_iters] = nc
    return _CACHED_NC[n_iters]


def host_tables():
    inv_freq = 1.0 / (ROPE_THETA ** (np.arange(0, HEAD_DIM, 2, dtype=np.float32) / HEAD_DIM))
    ang = np.arange(L, dtype=np.float32)[:, None] * inv_freq[None, :]  # [L, 64]
    cos, sin = np.cos(ang), np.sin(ang)
    cosT = np.concatenate([cos.T, cos.T], axis=0).astype(BF16_NP)
    sinT = np.concatenate([-sin.T, sin.T], axis=0).astype(BF16_NP)
    return np.ascontiguousarray(cosT), np.ascontiguousarray(sinT)


def host_mask():
    k = np.arange(128)[:, None]
    q = np.arange(512)[None, :]
    return np.ascontiguousarray((q >= k).astype(BF16_NP))


def make_in_maps(x, wq, wk, wv, wo):
    cosT, sinT = host_tables()
    mask = host_mask()
    xt = np.ascontiguousarray(x.reshape(L, D).T).astype(BF16_NP)
    in_maps = []
    for c in range(N_CORES):
        qs = slice(c * QH * 128, (c + 1) * QH * 128)
        kvs = slice(c * KVH * 128, (c + 1) * KVH * 128)
        in_maps.append({
            "xt": xt,
            "wqt": np.ascontiguousarray(wq[qs].T.astype(BF16_NP)),
            "wkt": np.ascontiguousarray(wk[kvs].T.astype(BF16_NP)),
            "wvt": np.ascontiguousarray(wv[kvs].T.astype(BF16_NP)),
            "wot": np.ascontiguousarray(wo[:, qs].T.astype(BF16_NP)),
            "cost": cosT,
            "sint": sinT,
            "mask": mask,
        })
    return in_maps


def run(inputs, trace=False, trace_kwargs=None):
    from concourse.bass_utils import run_bass_kernel_spmd

    nc = build()
    x = np.asarray(inputs["x"], dtype=np.float32)
    in_maps = make_in_maps(
        x,
        np.asarray(inputs["wq"], dtype=np.float32),
        np.asarray(inputs["wk"], dtype=np.float32),
        np.asarray(inputs["wv"], dtype=np.float32),
        np.asarray(inputs["wo"], dtype=np.float32),
    )
    res = run_bass_kernel_spmd(
        nc, in_maps, core_ids=list(range(N_CORES)),
        trace=trace, **(trace_kwargs or {}))
    out = np.zeros((L, D), dtype=np.float32)
    for c in range(N_CORES):
        out += res.results[c]["out"]
    return out.reshape(x.shape), res


def kernel(**inputs) -> np.ndarray:
    out, _ = run(inputs, trace=False)
    return out
